# revision 1
# baseline (speedup 1.0000x reference)
"""Trainium2 Bass kernel for nn_DetLoss_3762391351632.

Data-parallel over batch: 8 images -> 8 NeuronCores, one image per core.
Each core emits 5 partial scalars; host assembles & averages (the
"all-reduce mean" of the sharding hint, done on 8 floats).

Per-core algorithm (validated vs the reference in numpy, rel err ~3e-6):
  grid layout [128 partitions x 800]; partition p owns anchors
  [800p, 800p+800) for p<125; 3 pad partitions are masked out.
  - IoU in u-space: u = inter/(anchor_area+box_area), monotone in IoU;
    IoU thresholds 0.5/0.4 become u thresholds 1/3, 2/7.
  - per-anchor argmax over the 32 annotations and per-annotation argmax
    over all anchors via bit-packed quantized max (index in the low
    mantissa bits; bit30 forced on so packed values stay normal floats).
  - the reference's sequential low-quality-match scan == last-writer-wins
    scatter of <=32 values (dedup on a 32x32 tile + gpsimd local_scatter).
  - focal cls loss: sum over not-ignored rows of (1-a)*c^2*(-ln(1-c))
    via a PE trace trick: Square and Ln(1-c) as bf16 ACT outputs, their
    Frobenius inner product accumulated as 250 128x128 matmuls into one
    PSUM tile, diagonal extracted with a mask; plus exact per-anchor
    corrections at the assigned class.
  - annotation-field and class-column gathers are 32/40-way masked
    copy_predicated selects (no per-partition indexed gather exists).
"""
import math
import sys

sys.path.insert(0, "/opt/trn_rl_repo")

import numpy as np

import concourse.bass as bass
import concourse.bacc as bacc
import concourse.mybir as mybir
from concourse import bass_isa
from concourse.tile import TileContext

f32 = np.float32
dt = mybir.dt
ALU = mybir.AluOpType
ACTF = mybir.ActivationFunctionType
AX = mybir.AxisListType

A, M, C = 100000, 32, 40
P, PA, G = 128, 125, 800
NCHUNK = 10
GC = G // NCHUNK          # 80 anchors / partition / chunk
CHF = GC * C              # 3200 elems / partition / chunk
ALPHA = f32(0.25)
HI = float(f32(1.0 - 1e-4))
HI32 = f32(1.0 - 1e-4)
LO = float(f32(1e-4))
REG_W = f32(5.0)
T13 = float(f32(1.0 / 3.0))
T27 = float(f32(2.0 / 7.0))
BIT30 = 0x40000000
N_OUT = 8


def host_constants():
    g = np.arange(G, dtype=np.uint32)
    gcode = np.broadcast_to(((1023 - g) | BIT30)[None, :], (P, G)).copy()
    pio128 = np.broadcast_to(np.arange(P, dtype=f32)[None, :], (M, P)).copy()

    gio800 = np.broadcast_to(np.arange(G, dtype=f32)[None, :], (M, G)).copy()
    onesb = np.ones((1, P), dtype=f32)
    onesc = np.ones((P, 1), dtype=f32)
    jp1c = np.arange(1, M + 1, dtype=f32)[:, None]
    lt = (np.arange(M)[:, None] > np.arange(M)[None, :]).astype(f32)
    ident = np.eye(P, dtype=f32)
    vmask = (np.arange(P * G).reshape(P, G) < A).astype(f32)
    return {"gcode": gcode, "pio128": pio128, "gio800": gio800, "onesb": onesb,
            "onesc": onesc, "jp1c": jp1c, "ltmask": lt, "ident": ident,
            "vmask": vmask}


def build_bass(debug=False):
    nc = bacc.Bacc()
    cls_d = nc.declare_dram_parameter("classification", [P * G, C], dt.float32, isOutput=False)
    reg_d = nc.declare_dram_parameter("regression", [P * G, 4], dt.float32, isOutput=False)
    anc_d = nc.declare_dram_parameter("anchors", [P * G, 4], dt.float32, isOutput=False)
    ann_d = nc.declare_dram_parameter("annotation", [M, 5], dt.float32, isOutput=False)
    gcode_d = nc.declare_dram_parameter("gcode", [P, G], dt.uint32, isOutput=False)
    pio128_d = nc.declare_dram_parameter("pio128", [M, P], dt.float32, isOutput=False)
    gio800_d = nc.declare_dram_parameter("gio800", [M, G], dt.float32, isOutput=False)
    onesb_d = nc.declare_dram_parameter("onesb", [1, P], dt.float32, isOutput=False)
    onesc_d = nc.declare_dram_parameter("onesc", [P, 1], dt.float32, isOutput=False)
    jp1c_d = nc.declare_dram_parameter("jp1c", [M, 1], dt.float32, isOutput=False)
    lt_d = nc.declare_dram_parameter("ltmask", [M, M], dt.float32, isOutput=False)
    ident_d = nc.declare_dram_parameter("ident", [P, P], dt.float32, isOutput=False)
    vmask_d = nc.declare_dram_parameter("vmask", [P, G], dt.float32, isOutput=False)
    out_d = nc.declare_dram_parameter("out", [N_OUT], dt.float32, isOutput=True)
    dbg = {}
    if debug:
        for nm, shape, dty in [
            ("dbg_umaxq", [P, G], dt.float32), ("dbg_w0", [P, G], dt.float32),
            ("dbg_pos", [P, G], dt.float32), ("dbg_jeff", [P, G], dt.float32),
            ("dbg_csel", [P, G], dt.float32), ("dbg_colpk", [P, M], dt.uint32),
            ("dbg_rowpk", [P, G], dt.uint32), ("dbg_ovc", [P, G], dt.float32),
            ("dbg_u5", [P, G], dt.float32), ("dbg_clsg", [P, G], dt.float32),
            ("dbg_rsum", [P, G], dt.float32),
        ]:
            dbg[nm] = nc.declare_dram_parameter(nm, shape, dty, isOutput=True)

    v = nc.vector
    s = nc.scalar
    gp = nc.gpsimd
    te = nc.tensor

    with TileContext(nc) as tc:
        with (
            tc.tile_pool(name="const", bufs=1) as constp,
            tc.tile_pool(name="planes", bufs=1) as pl,
            tc.tile_pool(name="tmp", bufs=1) as tp,
            tc.tile_pool(name="chunks", bufs=2) as chp,
            tc.tile_pool(name="small", bufs=1) as sm,
            tc.tile_pool(name="smtmp", bufs=2) as st,
            tc.tile_pool(name="psum", bufs=2, space="PSUM") as pp,
        ):
            # ---------- constants ----------
            gcode = constp.tile([P, G], dt.uint32, name="gcode", tag="gcode")
            nc.sync.dma_start(gcode[:], gcode_d[:, :])
            pio128 = constp.tile([M, P], dt.float32, name="pio128", tag="pio128")
            nc.sync.dma_start(pio128[:], pio128_d[:, :])
            gio800 = constp.tile([M, G], dt.float32, name="gio800", tag="gio800")
            nc.sync.dma_start(gio800[:], gio800_d[:, :])
            onesb = constp.tile([1, P], dt.float32, name="onesb", tag="onesb")
            nc.sync.dma_start(onesb[:], onesb_d[:, :])
            onesc = constp.tile([P, 1], dt.float32, name="onesc", tag="onesc")
            nc.sync.dma_start(onesc[:], onesc_d[:, :])
            jp1c = constp.tile([M, 1], dt.float32, name="jp1c", tag="jp1c")
            nc.sync.dma_start(jp1c[:], jp1c_d[:, :])
            ltm = constp.tile([M, M], dt.float32, name="ltm", tag="ltm")
            nc.sync.dma_start(ltm[:], lt_d[:, :])
            ident = constp.tile([P, P], dt.float32, name="ident", tag="ident")
            nc.sync.dma_start(ident[:], ident_d[:, :])
            vmask = constp.tile([P, G], dt.float32, name="vmask", tag="vmask")
            nc.sync.dma_start(vmask[:], vmask_d[:, :])
            biasc = constp.tile([P, 2], dt.float32, name="biasc", tag="biasc")
            v.memset(biasc[:, 0:1], float(f32(math.pi / 2)))
            v.memset(biasc[:, 1:2], -1.0)


            def ts_bits(out_ap, in0_ap, s1, op0, s2=None, op1=None):
                ins = [v.lower_ap(in0_ap),
                       mybir.ImmediateValue(dtype=dt.uint32, value=int(s1))]
                if s2 is not None:
                    ins.append(mybir.ImmediateValue(dtype=dt.uint32, value=int(s2)))
                v.add_instruction(mybir.InstTensorScalarPtr(
                    name=nc.get_next_instruction_name(),
                    op0=op0, op1=(op1 if op1 is not None else ALU.bypass),
                    ins=ins, outs=[v.lower_ap(out_ap)]))

            def stt_bits(out_ap, in0_ap, s1, in1_ap, op0, op1):
                ins = [v.lower_ap(in0_ap),
                       mybir.ImmediateValue(dtype=dt.uint32, value=int(s1)),
                       v.lower_ap(in1_ap)]
                v.add_instruction(mybir.InstTensorScalarPtr(
                    name=nc.get_next_instruction_name(),
                    is_scalar_tensor_tensor=True,
                    op0=op0, op1=op1,
                    ins=ins, outs=[v.lower_ap(out_ap)]))

            # ---------- anchors ----------
            anc = pl.tile([P, 4 * G], dt.float32, name="anc", tag="anc")
            nc.sync.dma_start(anc[:, :], anc_d.rearrange("(p g) c -> p (g c)", p=P))
            x1 = anc[:, 0:4 * G:4]
            y1 = anc[:, 1:4 * G:4]
            x2 = anc[:, 2:4 * G:4]
            y2 = anc[:, 3:4 * G:4]

            aa = pl.tile([P, G], dt.float32, name="aa", tag="aa")
            aw = pl.tile([P, G], dt.float32, name="aw", tag="aw")
            ah = pl.tile([P, G], dt.float32, name="ah", tag="ah")
            v.tensor_tensor(aw[:], x2, x1, op=ALU.subtract)
            v.tensor_tensor(ah[:], y2, y1, op=ALU.subtract)
            v.tensor_tensor(aa[:], aw[:], ah[:], op=ALU.mult)

            # ---------- annotation prep (rows on partitions 0..4) ----------
            annT = sm.tile([1, 5 * M], dt.float32, name="annT", tag="annT")
            with nc.allow_non_contiguous_dma(reason="tiny 32x5 transposed load"):
                nc.sync.dma_start(annT[:].rearrange("o (f m) -> o f m", m=M), ann_d.rearrange("m f -> f m")[None, :, :])
            cxr, cyr, thr, lnr, clsr = (annT[:, i * M:(i + 1) * M] for i in range(5))

            valid_r = sm.tile([1, M], dt.float32, name="valid", tag="valid")
            v.tensor_scalar(valid_r[:], clsr, -1.0, None, op0=ALU.not_equal)
            wk = lambda tag: st.tile([1, M], dt.float32, name=tag, tag=tag)
            cosv, sinv, dxv, dyv = wk("cosv"), wk("sinv"), wk("dxv"), wk("dyv")
            s.activation(cosv[:], thr, ACTF.Sin, bias=biasc[0:1, 0:1], scale=-1.0)
            s.activation(sinv[:], thr, ACTF.Sin)
            t0 = wk("t0")
            v.tensor_tensor(t0[:], lnr, cosv[:], op=ALU.mult)
            s.activation(dxv[:], t0[:], ACTF.Abs, scale=0.5)
            v.tensor_tensor(t0[:], lnr, sinv[:], op=ALU.mult)
            s.activation(dyv[:], t0[:], ACTF.Abs, scale=0.5)
            v.tensor_tensor(dxv[:], dxv[:], valid_r[:], op=ALU.mult)
            v.tensor_tensor(dyv[:], dyv[:], valid_r[:], op=ALU.mult)

            # per-j scalar bundle -> broadcast [P, 7M]:
            # cols: 0 bx1, 1 negbx1, 2 bw, 3 by1, 4 negby1, 5 bh, 6 ar4
            bsrc = sm.tile([1, 7 * M], dt.float32, name="bsrc", tag="bsrc")
            v.tensor_tensor(bsrc[:, 0 * M:1 * M], cxr, dxv[:], op=ALU.subtract)
            v.tensor_scalar(bsrc[:, 1 * M:2 * M], bsrc[:, 0 * M:1 * M], -1.0, None, op0=ALU.mult)
            v.tensor_scalar(bsrc[:, 2 * M:3 * M], dxv[:], 2.0, None, op0=ALU.mult)
            v.tensor_tensor(bsrc[:, 3 * M:4 * M], cyr, dyv[:], op=ALU.subtract)
            v.tensor_scalar(bsrc[:, 4 * M:5 * M], bsrc[:, 3 * M:4 * M], -1.0, None, op0=ALU.mult)
            v.tensor_scalar(bsrc[:, 5 * M:6 * M], dyv[:], 2.0, None, op0=ALU.mult)
            v.scalar_tensor_tensor(bsrc[:, 6 * M:7 * M], dxv[:], 4.0, dyv[:], op0=ALU.mult, op1=ALU.mult)
            BC_ps = pp.tile([P, 7 * M], dt.float32, name="BC_ps", tag="ps_s")
            te.matmul(BC_ps[:], onesb[:], bsrc[:], start=True, stop=True)
            BC = sm.tile([P, 7 * M], dt.float32, name="BC", tag="BC")
            s.copy(BC[:], BC_ps[:])
            col = lambda f, j: BC[:, f * M + j:f * M + j + 1]

            # select tables -> broadcast [P, 5M]: cx, cy, th, lnl, cls
            tsrc = sm.tile([1, 5 * M], dt.float32, name="tsrc", tag="tsrc")
            v.tensor_copy(tsrc[:, 0 * M:1 * M], cxr)
            v.tensor_copy(tsrc[:, 1 * M:2 * M], cyr)
            v.tensor_copy(tsrc[:, 2 * M:3 * M], thr)
            lnmx = wk("lnmx")
            v.tensor_scalar(lnmx[:], lnr, 1.0, None, op0=ALU.max)
            s.activation(tsrc[:, 3 * M:4 * M], lnmx[:], ACTF.Ln)
            v.tensor_copy(tsrc[:, 4 * M:5 * M], clsr)
            TBL_ps = pp.tile([P, 5 * M], dt.float32, name="TBL_ps", tag="ps_s")
            te.matmul(TBL_ps[:], onesb[:], tsrc[:], start=True, stop=True)
            TBL = sm.tile([P, 5 * M], dt.float32, name="TBL", tag="TBL")
            s.copy(TBL[:], TBL_ps[:])

            # ---------- IoU loop ----------
            rowpk = pl.tile([P, G], dt.float32, name="rowpk", tag="rowpk")
            v.memset(rowpk[:], 0.0)
            colpk = pl.tile([P, M], dt.float32, name="colpk", tag="colpk")

            for j in range(M):
                rx = tp.tile([P, G], dt.float32, name="t_rx", tag="tA")
                s.activation(rx[:], x1, ACTF.Relu, bias=col(1, j))
                iw = tp.tile([P, G], dt.float32, name="t_iw", tag="tB")
                v.tensor_scalar(iw[:], x2, col(0, j), col(2, j), op0=ALU.subtract, op1=ALU.min)
                v.tensor_tensor(iw[:], iw[:], rx[:], op=ALU.subtract)
                ry = tp.tile([P, G], dt.float32, name="t_ry", tag="tA")
                s.activation(ry[:], y1, ACTF.Relu, bias=col(4, j))
                ih = tp.tile([P, G], dt.float32, name="t_ih", tag="tC")
                v.tensor_scalar(ih[:], y2, col(3, j), col(5, j), op0=ALU.subtract, op1=ALU.min)
                v.tensor_tensor(ih[:], ih[:], ry[:], op=ALU.subtract)
                v.tensor_scalar(ih[:], ih[:], 0.0, None, op0=ALU.max)
                inter = tp.tile([P, G], dt.float32, name="t_inter", tag="tD")
                v.scalar_tensor_tensor(inter[:], iw[:], 0.0, ih[:], op0=ALU.max, op1=ALU.mult)
                S = tp.tile([P, G], dt.float32, name="t_S", tag="tE")
                v.tensor_scalar(S[:], aa[:], col(6, j), 1e-8, op0=ALU.add, op1=ALU.max)
                r = tp.tile([P, G], dt.float32, name="t_r", tag="tF")
                v.reciprocal(r[:], S[:])
                u = tp.tile([P, G], dt.float32, name="t_u", tag="tG")
                v.tensor_tensor(u[:], inter[:], r[:], op=ALU.mult)
                if debug and j == 5:
                    nc.sync.dma_start(dbg["dbg_u5"][:, :], u[:])
                ub = u[:].bitcast(dt.uint32)
                gpk = tp.tile([P, G], dt.uint32, name="t_gpk", tag="tH")
                stt_bits(gpk[:], ub, 0xFFFFFC00, gcode[:], op0=ALU.bitwise_and, op1=ALU.bitwise_or)
                v.tensor_reduce(colpk[:, j:j + 1], gpk[:].bitcast(dt.float32), axis=AX.X, op=ALU.max)
                jpk = tp.tile([P, G], dt.uint32, name="t_jpk", tag="tH")
                ts_bits(jpk[:], ub, 0xFFFFFFE0, op0=ALU.bitwise_and, s2=(31 - j) | BIT30, op1=ALU.bitwise_or)
                v.tensor_tensor(rowpk[:], rowpk[:], jpk[:].bitcast(dt.float32), op=ALU.max)

            # ---------- per-anchor decode ----------
            umaxq = pl.tile([P, G], dt.float32, name="umaxq", tag="umaxq")
            ts_bits(umaxq[:].bitcast(dt.uint32), rowpk[:].bitcast(dt.uint32), 0xBFFFFFE0, op0=ALU.bitwise_and)
            jstar = pl.tile([P, G], dt.float32, name="jstar", tag="jstar")
            wst = tp.tile([P, G], dt.uint32, name="t_wst", tag="tH")
            ts_bits(wst[:], rowpk[:].bitcast(dt.uint32), 0x1F, op0=ALU.bitwise_and)
            v.tensor_copy(jstar[:], wst[:])
            v.tensor_scalar(jstar[:], jstar[:], -1.0, 31.0, op0=ALU.mult, op1=ALU.add)
            ge13 = pl.tile([P, G], dt.float32, name="ge13", tag="ge13")
            v.tensor_scalar(ge13[:], umaxq[:], T13, None, op0=ALU.is_ge)
            ge27 = pl.tile([P, G], dt.float32, name="ge27", tag="ge27")
            v.tensor_scalar(ge27[:], umaxq[:], T27, None, op0=ALU.is_ge)

            # ---------- column stats ----------
            cpT_ps = pp.tile([M, P], dt.float32, name="cpT", tag="ps_s")
            te.transpose(cpT_ps[:], colpk[:], ident[:])
            cpT = sm.tile([M, P], dt.float32, name="cpTs", tag="cpTs")
            s.copy(cpT[:], cpT_ps[:])
            mx8 = sm.tile([M, 8], dt.float32, name="mx8", tag="mx8")
            v.max(mx8[:], cpT[:])
            mi8 = sm.tile([M, 8], dt.uint32, name="mi8", tag="mi8")
            v.max_index(mi8[:], mx8[:], cpT[:])

            bun = sm.tile([M, 4], dt.float32, name="bun", tag="bun")
            v.tensor_copy(bun[:, 0:1], mi8[:, 0:1])                  # pstar
            pkb = mx8[:, 0:1].bitcast(dt.uint32)
            g10u = st.tile([M, 1], dt.uint32, name="g10u", tag="g10u")
            ts_bits(g10u[:], pkb, 0x3FF, op0=ALU.bitwise_and)
            v.tensor_copy(bun[:, 1:2], g10u[:])
            v.tensor_scalar(bun[:, 1:2], bun[:, 1:2], -1.0, 1023.0, op0=ALU.mult, op1=ALU.add)  # gstar
            ts_bits(bun[:, 2:3].bitcast(dt.uint32), pkb, 0xBFFFFC00, op0=ALU.bitwise_and)
            acol = st.tile([M, 1], dt.float32, name="acol", tag="acol")
            v.scalar_tensor_tensor(acol[:], bun[:, 0:1], 800.0, bun[:, 1:2], op0=ALU.mult, op1=ALU.add)
            docol = st.tile([M, 1], dt.float32, name="docol", tag="docol")
            v.tensor_scalar(docol[:], bun[:, 2:3], T13, None, op0=ALU.is_lt)
            validc_ps = pp.tile([M, 1], dt.float32, name="validc", tag="ps_s")
            te.transpose(validc_ps[:], valid_r[:], ident[0:1, 0:1])
            validc = st.tile([M, 1], dt.float32, name="validc_sb", tag="validc_sb")
            s.copy(validc[:], validc_ps[:])
            v.tensor_tensor(docol[:], docol[:], validc[:], op=ALU.mult)
            v.tensor_copy(bun[:, 3:4], docol[:])

            # vscat (column form): do * (j+1) * not-killed, dedup last-wins
            # kill_k = sum_l>k (a_l == a_k) * do_l >= 1, via PE ones-reduction
            arow_ps = pp.tile([1, M], dt.float32, name="arow_ps", tag="ps_s")
            te.transpose(arow_ps[:], acol[:], ident[:M, :M])
            arow = st.tile([1, M], dt.float32, name="arow", tag="arow")
            s.copy(arow[:], arow_ps[:])
            abc_ps = pp.tile([M, M], dt.float32, name="abc_ps", tag="ps_s")
            te.matmul(abc_ps[:], onesb[:, :M], arow[:], start=True, stop=True)
            eqm = sm.tile([M, M], dt.float32, name="eqm", tag="eqm")
            v.tensor_tensor(eqm[:], abc_ps[:], acol[:].broadcast_to((M, M)), op=ALU.is_equal)
            v.tensor_tensor(eqm[:], eqm[:], docol[:].broadcast_to((M, M)), op=ALU.mult)
            v.tensor_tensor(eqm[:], eqm[:], ltm[:], op=ALU.mult)
            killc_ps = pp.tile([M, 1], dt.float32, name="killc_ps", tag="ps_s")
            te.matmul(killc_ps[:], eqm[:], onesc[:M, :], start=True, stop=True)
            vscat_c = st.tile([M, 1], dt.float32, name="vscat_c", tag="vscat_c")
            v.tensor_scalar(vscat_c[:], killc_ps[:], 1.0, None, op0=ALU.is_lt)
            v.tensor_tensor(vscat_c[:], vscat_c[:], docol[:], op=ALU.mult)
            v.tensor_tensor(vscat_c[:], vscat_c[:], jp1c[:], op=ALU.mult)

            # override plane via rank-32 PE outer product:
            # ovc[p,g] = sum_j vscat_j * (p==pstar_j) * (g==gstar_j)
            Lm = sm.tile([M, P], dt.float32, name="Lm", tag="Lm")
            v.tensor_tensor(Lm[:], pio128[:], bun[:, 0:1].broadcast_to((M, P)), op=ALU.is_equal)
            v.tensor_tensor(Lm[:], Lm[:], vscat_c[:].broadcast_to((M, P)), op=ALU.mult)
            Rm = sm.tile([M, G], dt.float32, name="Rm", tag="Rm")
            v.tensor_tensor(Rm[:], gio800[:], bun[:, 1:2].broadcast_to((M, G)), op=ALU.is_equal)
            ovc_ps = pp.tile([P, G], dt.float32, name="ovc_ps", tag="ovc_ps", bufs=1)
            te.matmul(ovc_ps[:, 0:512], Lm[:], Rm[:, 0:512], start=True, stop=True)
            te.matmul(ovc_ps[:, 512:G], Lm[:], Rm[:, 512:G], start=True, stop=True)
            ovc = tp.tile([P, G], dt.float32, name="t_ovc", tag="tB")
            s.copy(ovc[:], ovc_ps[:])
            ovf = pl.tile([P, G], dt.float32, name="ovf", tag="ovf")
            v.tensor_scalar(ovf[:], ovc[:], 0.0, None, op0=ALU.is_gt)

            jeff = pl.tile([P, G], dt.float32, name="jeff", tag="jeff")
            v.tensor_copy(jeff[:], jstar[:])
            ovj = tp.tile([P, G], dt.float32, name="t_ovj", tag="tC")
            v.tensor_scalar(ovj[:], ovc[:], 1.0, None, op0=ALU.subtract)
            ovf8 = tp.tile([P, G], dt.uint8, name="t_ovf8", tag="tD")
            v.tensor_copy(ovf8[:], ovf[:])
            v.copy_predicated(jeff[:], ovf8[:], ovj[:])

            pos = pl.tile([P, G], dt.float32, name="pos", tag="pos")
            v.tensor_tensor(pos[:], ge13[:], ovf[:], op=ALU.max)
            v.tensor_tensor(pos[:], pos[:], vmask[:], op=ALU.mult)
            w0 = pl.tile([P, G], dt.float32, name="w0", tag="w0")
            v.tensor_tensor(w0[:], ge27[:], ge13[:], op=ALU.subtract)
            nov = tp.tile([P, G], dt.float32, name="t_nov", tag="tD")
            v.tensor_scalar(nov[:], ovf[:], -1.0, 1.0, op0=ALU.mult, op1=ALU.add)
            v.tensor_tensor(w0[:], w0[:], nov[:], op=ALU.mult)
            v.tensor_scalar(w0[:], w0[:], -1.0, 1.0, op0=ALU.mult, op1=ALU.add)
            v.tensor_tensor(w0[:], w0[:], vmask[:], op=ALU.mult)

            # ---------- 32-way field select (gather replacement) ----------
            cxg = pl.tile([P, G], dt.float32, name="cxg", tag="cxg")
            cyg = pl.tile([P, G], dt.float32, name="cyg", tag="cyg")
            thg = pl.tile([P, G], dt.float32, name="thg", tag="thg")
            lnlg = pl.tile([P, G], dt.float32, name="lnlg", tag="lnlg")
            clsg = pl.tile([P, G], dt.float32, name="clsg", tag="clsg")
            fields = [cxg, cyg, thg, lnlg, clsg]
            for fi, dst in enumerate(fields):
                v.tensor_copy(dst[:], TBL[:, fi * M:fi * M + 1].broadcast_to((P, G)))
            for j in range(1, M):
                mj = tp.tile([P, G], dt.uint8, name="t_mj", tag="tA")
                v.tensor_scalar(mj[:], jeff[:], float(j), None, op0=ALU.is_equal)
                for fi, dst in enumerate(fields):
                    v.copy_predicated(dst[:], mj[:], TBL[:, fi * M + j:fi * M + j + 1].broadcast_to((P, G)))

            inR = tp.tile([P, G], dt.float32, name="t_inr", tag="tB")
            v.tensor_scalar(inR[:], clsg[:], 0.0, None, op0=ALU.is_ge)
            inR2 = tp.tile([P, G], dt.float32, name="t_inr2", tag="tC")
            v.tensor_scalar(inR2[:], clsg[:], float(C - 1), None, op0=ALU.is_le)
            v.tensor_tensor(inR[:], inR[:], inR2[:], op=ALU.mult)
            v.tensor_tensor(pos[:], pos[:], inR[:], op=ALU.mult)
            kstar = pl.tile([P, G], dt.float32, name="kstar", tag="kstar")
            v.tensor_scalar(kstar[:], clsg[:], float(C - 1), 0.0, op0=ALU.min, op1=ALU.max)

            if debug:
                nc.sync.dma_start(dbg["dbg_umaxq"][:, :], umaxq[:])
                nc.sync.dma_start(dbg["dbg_w0"][:, :], w0[:])
                nc.sync.dma_start(dbg["dbg_pos"][:, :], pos[:])
                nc.sync.dma_start(dbg["dbg_jeff"][:, :], jeff[:])
                nc.sync.dma_start(dbg["dbg_colpk"][:, :], colpk[:].bitcast(dt.uint32))
                nc.sync.dma_start(dbg["dbg_rowpk"][:, :], rowpk[:].bitcast(dt.uint32))
                nc.sync.dma_start(dbg["dbg_ovc"][:, :], ovc[:])
                nc.sync.dma_start(dbg["dbg_clsg"][:, :], clsg[:])

            # k-masks for the csel select (40 planes would be too much SBUF;
            # compute one [P,G] mask per k on the fly inside the chunk loop
            # would redo work 10x; instead compute all 40 as uint8? -> keep
            # it simple: compute mask per (k) once into a temp and apply to
            # all 10 chunk slices immediately (csel is chunk-agnostic: data
            # comes from the chunk tile, mask from the full plane slice).

            # ---------- [A,C] streaming: trace + csel ----------
            cselb = pl.tile([P, G], dt.float32, name="cselb", tag="cselb")
            v.memset(cselb[:], 0.0)
            tracep = pp.tile([P, P], dt.float32, name="trace", tag="trace", bufs=1)
            clsv = cls_d.rearrange("(p g) c -> p (g c)", p=P)
            for ci in range(NCHUNK):
                cr = chp.tile([P, CHF], dt.float32, name="cr", tag="cr")
                nc.sync.dma_start(cr[:, :], clsv[:, ci * CHF:(ci + 1) * CHF])
                ctf = chp.tile([P, CHF], dt.float32, name="ctf", tag="ctf")
                v.tensor_scalar(ctf[:], cr[:], HI, None, op0=ALU.min)
                sq = chp.tile([P, CHF], dt.bfloat16, name="sq", tag="sq")
                s.activation(sq[:], ctf[:], ACTF.Square)
                lg = chp.tile([P, CHF], dt.bfloat16, name="lg", tag="lg")
                s.activation(lg[:], ctf[:], ACTF.Ln, bias=1.0, scale=-1.0)
                w0b = w0[:, ci * GC:(ci + 1) * GC].unsqueeze(-1).broadcast_to((P, GC, C))
                v.tensor_tensor(sq[:].rearrange("p (g c) -> p g c", c=C),
                                sq[:].rearrange("p (g c) -> p g c", c=C), w0b, op=ALU.mult)
                for mi in range(CHF // P):
                    te.matmul(tracep[:], sq[:, mi * P:(mi + 1) * P], lg[:, mi * P:(mi + 1) * P],
                              start=(ci == 0 and mi == 0), stop=(ci == NCHUNK - 1 and mi == CHF // P - 1))
                # csel for this chunk: 40-way class-column select
                ctv = ctf[:].rearrange("p (g c) -> p g c", c=C)
                for k in range(C):
                    mk_ = st.tile([P, GC], dt.uint8, name="t_mk", tag="t_mk")
                    v.tensor_scalar(mk_[:], kstar[:, ci * GC:(ci + 1) * GC], float(k), None, op0=ALU.is_equal)
                    v.copy_predicated(cselb[:, ci * GC:(ci + 1) * GC], mk_[:], ctv[:, :, k])

            trsb = tp.tile([P, P], dt.float32, name="t_trash", tag="t_trash")
            s.copy(trsb[:], tracep[:])
            v.tensor_tensor(trsb[:], trsb[:], ident[:], op=ALU.mult)
            dsum = sm.tile([P, 1], dt.float32, name="dsum", tag="dsum")
            v.tensor_reduce(dsum[:], trsb[:], axis=AX.X, op=ALU.add)

            csel = cselb
            if debug:
                nc.sync.dma_start(dbg["dbg_csel"][:, :], csel[:])

            # ---------- delta terms ----------
            acc = sm.tile([P, 4], dt.float32, name="acc", tag="acc")
            cselLO = tp.tile([P, G], dt.float32, name="t_cslo", tag="tA")
            v.tensor_scalar(cselLO[:], csel[:], LO, None, op0=ALU.max)
            lnc = tp.tile([P, G], dt.float32, name="t_lnc", tag="tB")
            s.activation(lnc[:], cselLO[:], ACTF.Ln)
            ln1c = tp.tile([P, G], dt.float32, name="t_ln1c", tag="tC")
            s.activation(ln1c[:], csel[:], ACTF.Ln, bias=1.0, scale=-1.0)
            om2 = tp.tile([P, G], dt.float32, name="t_om2", tag="tD")
            v.tensor_scalar(om2[:], csel[:], -1.0, 1.0, op0=ALU.mult, op1=ALU.add)
            v.tensor_tensor(om2[:], om2[:], om2[:], op=ALU.mult)
            c2 = tp.tile([P, G], dt.float32, name="t_c2", tag="tE")
            v.tensor_tensor(c2[:], csel[:], csel[:], op=ALU.mult)
            v.tensor_tensor(om2[:], om2[:], lnc[:], op=ALU.mult)
            v.scalar_tensor_tensor(om2[:], om2[:], 1.0, pos[:], op0=ALU.mult, op1=ALU.mult, accum_out=acc[:, 0:1])
            v.tensor_tensor(c2[:], c2[:], ln1c[:], op=ALU.mult)
            v.scalar_tensor_tensor(c2[:], c2[:], 1.0, pos[:], op0=ALU.mult, op1=ALU.mult, accum_out=acc[:, 1:2])
            npt = tp.tile([P, G], dt.float32, name="t_npt", tag="tF")
            v.tensor_scalar(npt[:], pos[:], 0.0, 0.0, op0=ALU.add, op1=ALU.add, accum_out=acc[:, 2:3])

            # ---------- regression ----------
            regr = chp.tile([P, 4 * G], dt.float32, name="cr", tag="cr")
            nc.sync.dma_start(regr[:, :], reg_d.rearrange("(p g) c -> p (g c)", p=P))
            reg0 = regr[:, 0:4 * G:4]
            reg1 = regr[:, 1:4 * G:4]
            reg2 = regr[:, 2:4 * G:4]
            reg3 = regr[:, 3:4 * G:4]

            rw2 = tp.tile([P, G], dt.float32, name="t_rw2", tag="tA")
            v.reciprocal(rw2[:], aw[:])
            rh2 = tp.tile([P, G], dt.float32, name="t_rh2", tag="tB")
            v.reciprocal(rh2[:], ah[:])
            lnal = tp.tile([P, G], dt.float32, name="t_lnal", tag="tC")
            aw2 = tp.tile([P, G], dt.float32, name="t_aw2", tag="tD")
            v.tensor_tensor(aw2[:], aw[:], aw[:], op=ALU.mult)
            ah2 = tp.tile([P, G], dt.float32, name="t_ah2", tag="tE")
            v.tensor_tensor(ah2[:], ah[:], ah[:], op=ALU.mult)
            v.tensor_tensor(aw2[:], aw2[:], ah2[:], op=ALU.add)
            s.activation(lnal[:], aw2[:], ACTF.Ln)

            rsum = pl.tile([P, G], dt.float32, name="rsum", tag="rsum")
            dtl = tp.tile([P, G], dt.float32, name="t_dtl", tag="tF")
            dd = tp.tile([P, G], dt.float32, name="t_dd", tag="tG")

            def sl1_accum(first):
                m_ = tp.tile([P, G], dt.float32, name="t_sl1m", tag="tD")
                v.tensor_scalar(m_[:], dd[:], 1.0, None, op0=ALU.min)
                v.tensor_tensor(m_[:], m_[:], m_[:], op=ALU.mult)
                rl_ = tp.tile([P, G], dt.float32, name="t_sl1r", tag="tE")
                s.activation(rl_[:], dd[:], ACTF.Relu, bias=biasc[:, 1:2])
                if first:
                    v.scalar_tensor_tensor(rsum[:], m_[:], 0.5, rl_[:], op0=ALU.mult, op1=ALU.add)
                else:
                    v.scalar_tensor_tensor(m_[:], m_[:], 0.5, rl_[:], op0=ALU.mult, op1=ALU.add)
                    v.tensor_tensor(rsum[:], rsum[:], m_[:], op=ALU.add)

            # d0: |(cxg - (x1+x2)/2) * 2/aw - reg0|
            v.tensor_tensor(dtl[:], x1, x2, op=ALU.add)
            v.tensor_scalar(dtl[:], dtl[:], 0.5, None, op0=ALU.mult)
            v.tensor_tensor(dtl[:], cxg[:], dtl[:], op=ALU.subtract)
            v.tensor_tensor(dtl[:], dtl[:], rw2[:], op=ALU.mult)
            v.tensor_scalar(dtl[:], dtl[:], 2.0, None, op0=ALU.mult)
            v.tensor_tensor(dtl[:], dtl[:], reg0, op=ALU.subtract)
            s.activation(dd[:], dtl[:], ACTF.Abs)
            sl1_accum(True)
            # d1
            v.tensor_tensor(dtl[:], y1, y2, op=ALU.add)
            v.tensor_scalar(dtl[:], dtl[:], 0.5, None, op0=ALU.mult)
            v.tensor_tensor(dtl[:], cyg[:], dtl[:], op=ALU.subtract)
            v.tensor_tensor(dtl[:], dtl[:], rh2[:], op=ALU.mult)
            v.tensor_scalar(dtl[:], dtl[:], 2.0, None, op0=ALU.mult)
            v.tensor_tensor(dtl[:], dtl[:], reg1, op=ALU.subtract)
            s.activation(dd[:], dtl[:], ACTF.Abs)
            sl1_accum(False)
            # d2: |sin(thg - reg2)| with range reduction into (-pi, pi]
            v.tensor_tensor(dtl[:], thg[:], reg2, op=ALU.subtract)
            TWO_PI = float(f32(2.0 * math.pi))
            PI_ = float(f32(math.pi))
            gtpi = tp.tile([P, G], dt.float32, name="gtpi", tag="tA")
            for _ in range(2):
                v.tensor_scalar(gtpi[:], dtl[:], PI_, None, op0=ALU.is_gt)
                v.scalar_tensor_tensor(dtl[:], gtpi[:], -TWO_PI, dtl[:], op0=ALU.mult, op1=ALU.add)
            v.tensor_scalar(gtpi[:], dtl[:], -PI_, None, op0=ALU.is_lt)
            v.scalar_tensor_tensor(dtl[:], gtpi[:], TWO_PI, dtl[:], op0=ALU.mult, op1=ALU.add)
            s.activation(dtl[:], dtl[:], ACTF.Sin)
            s.activation(dd[:], dtl[:], ACTF.Abs)
            sl1_accum(False)
            # d3: |2*(lnlg - 0.5*ln(aw^2+ah^2)) - reg3|
            v.scalar_tensor_tensor(dtl[:], lnal[:], 0.5, lnlg[:], op0=ALU.mult, op1=ALU.subtract)
            v.tensor_scalar(dtl[:], dtl[:], -2.0, None, op0=ALU.mult)
            v.tensor_tensor(dtl[:], dtl[:], reg3, op=ALU.subtract)
            s.activation(dd[:], dtl[:], ACTF.Abs)
            sl1_accum(False)

            if debug:
                nc.sync.dma_start(dbg["dbg_rsum"][:, :], rsum[:])
            v.scalar_tensor_tensor(rsum[:], rsum[:], 1.0, pos[:], op0=ALU.mult, op1=ALU.mult,
                                   accum_out=acc[:, 3:4])

            # ---------- final reduction (PE ones-matmul over partitions) ----------
            accr_ps = pp.tile([1, 4], dt.float32, name="accr_ps", tag="ps_s")
            te.matmul(accr_ps[:], onesc[:], acc[:], start=True, stop=True)
            dsr_ps = pp.tile([1, 1], dt.float32, name="dsr_ps", tag="ps_s")
            te.matmul(dsr_ps[:], onesc[:], dsum[:], start=True, stop=True)
            outsb = sm.tile([1, N_OUT], dt.float32, name="outsb", tag="outsb")
            v.memset(outsb[:], 0.0)
            v.tensor_copy(outsb[:, 0:1], dsr_ps[:])
            v.tensor_copy(outsb[:, 1:5], accr_ps[:])
            nc.sync.dma_start(out_d[None, :], outsb[:])
    nc.finalize()
    return nc


_CACHED = {}


def _get_nc(debug=False):
    key = bool(debug)
    if key not in _CACHED:
        _CACHED[key] = build_bass(debug=key)
    return _CACHED[key]


def assemble(outs):
    cls_l, reg_l = [], []
    for o in outs:
        o0, o1, o2, o3, o4 = (f32(o[i]) for i in range(5))
        np1 = max(o3, f32(1.0))
        cls_l.append((-(f32(1.0) - ALPHA) * (o0 - o2) - ALPHA * o1) / np1)
        reg_l.append(REG_W * o4 / np1)
    return f32(np.mean(np.array(cls_l, dtype=f32)) + np.mean(np.array(reg_l, dtype=f32)))


def make_in_maps(classifications, regressions, anchors_pos, annotations):
    consts = host_constants()
    anc_pad = np.empty((P * G, 4), dtype=f32)
    anc_pad[:A] = anchors_pos
    anc_pad[A:, 0] = anc_pad[A:, 1] = -2.0e6
    anc_pad[A:, 2] = anc_pad[A:, 3] = -1.0e6
    in_maps = []
    for b in range(classifications.shape[0]):
        cls_pad = np.full((P * G, C), 0.5, dtype=f32)
        cls_pad[:A] = classifications[b]
        reg_pad = np.zeros((P * G, 4), dtype=f32)
        reg_pad[:A] = regressions[b]
        m = {
            "classification": cls_pad,
            "regression": reg_pad,
            "anchors": anc_pad,
            "annotation": np.ascontiguousarray(annotations[b], dtype=np.float32),
        }
        m.update(consts)
        in_maps.append(m)
    return in_maps


def kernel(classifications, regressions, anchors_pos, annotations):
    from concourse.bass_utils import run_bass_kernel_spmd
    nc = _get_nc(debug=False)
    in_maps = make_in_maps(classifications, regressions, anchors_pos, annotations)
    res = run_bass_kernel_spmd(nc, in_maps, list(range(classifications.shape[0])))
    outs = [res.results[b]["out"] for b in range(classifications.shape[0])]
    return np.array(assemble(outs), dtype=np.float32)

